# revision 16
# baseline (speedup 1.0000x reference)
"""EnsemblePooling (segment mean/max/attention pooling) on 8 Trainium2 cores.

Contract: kernel(**inputs) takes the FULL inputs (x [N,256] f32,
batch [N] i64 sorted, att_w [256,1] f32, att_b [1] f32) and returns the
FULL output [1024, 768] f32 = concat([mean_pool, max_pool, att_pool], -1).

Strategy (all hardcoded, self-contained):
  - core c owns segments [128c, 128(c+1)); nodes are sharded by segment.
  - host pads every segment's node run to a multiple of 128 so each
    128-node tile belongs to exactly ONE segment -> a single SPMD
    program works for all cores; per-core differences are pure data.
  - x is shipped bf16 (halves HBM traffic; PSUM accumulation stays f32).
  - per tile: one-hot(batch_local) routes the tile's rows into the
    right PSUM partition via accumulating matmuls (segment sum and
    sigmoid-weighted sum); PE transposes the tile so DVE can reduce
    max along the free dim into per-tile max columns (interleaved
    (tile, hidden-chunk) layout, one fused reduce per tile pair).
  - epilogue: masked max tournament folds per-tile max columns over
    each segment's tile run; one-hot extraction matmuls move the
    per-segment max back to [seg, hidden] layout.
"""

import numpy as np

P = 128
H = 256
G = 1024
CORES = 8
SEGS_PER_CORE = G // CORES  # 128
PAD_X = -1.0e20
NEG_BIG = -1.5e38
S_TILES = 8  # node-tiles per DMA super-tile

_compiled_cache = {}


def _bf16(arr):
    import ml_dtypes

    return np.asarray(arr).astype(ml_dtypes.bfloat16)


def _build_program(NT, KC, ks):
    import concourse.bacc as bacc
    import concourse.tile as tile
    from concourse import mybir

    f32 = mybir.dt.float32
    bf16 = mybir.dt.bfloat16
    NTpad = KC * P
    KC2 = (2 * NT + P - 1) // P  # chunks over interleaved (tile, chunk) cols
    NC2pad = KC2 * P

    nc = bacc.Bacc("TRN2", target_bir_lowering=False, debug=False)

    x_d = nc.declare_dram_parameter("x", [P, NT, H], bf16, isOutput=False)
    bl_d = nc.declare_dram_parameter("bl", [P, NT], f32, isOutput=False)
    wrow2_d = nc.declare_dram_parameter("wrow2", [P, 2 * H], bf16, isOutput=False)
    bcol_d = nc.declare_dram_parameter("bcol", [P, 1], f32, isOutput=False)
    iota_d = nc.declare_dram_parameter("iota", [P, P], bf16, isOutput=False)
    ident_d = nc.declare_dram_parameter("ident", [P, P], bf16, isOutput=False)
    ohm0_d = nc.declare_dram_parameter("ohm0", [P, KC2, P], f32, isOutput=False)
    ohm1_d = nc.declare_dram_parameter("ohm1", [P, KC2, P], f32, isOutput=False)
    bias_d = {
        k: nc.declare_dram_parameter(f"bias{k}", [P, 2 * NT], f32, isOutput=False)
        for k in ks
    }
    invcnt_d = nc.declare_dram_parameter("invcnt", [P, 1], f32, isOutput=False)
    out_d = nc.declare_dram_parameter("out", [P, 3 * H], f32, isOutput=True)

    with (
        tile.TileContext(nc) as tc,
        tc.tile_pool(name="const", bufs=1) as cpool,
        tc.tile_pool(name="xp", bufs=3) as xpool,
        tc.tile_pool(name="work", bufs=3) as wpool,
        tc.tile_pool(name="acc", bufs=1, space="PSUM") as apool,
        tc.tile_pool(name="pst", bufs=2, space="PSUM") as tpool,
    ):
        # persistent constants
        wrow2 = cpool.tile([P, 2 * H], bf16)
        nc.sync.dma_start(out=wrow2[:], in_=wrow2_d[:])
        bcol = cpool.tile([P, 1], f32)
        nc.sync.dma_start(out=bcol[:], in_=bcol_d[:])
        iota = cpool.tile([P, P], bf16)
        nc.sync.dma_start(out=iota[:], in_=iota_d[:])
        ident = cpool.tile([P, P], bf16)
        nc.sync.dma_start(out=ident[:], in_=ident_d[:])
        bl = cpool.tile([P, NT], f32)
        nc.sync.dma_start(out=bl[:], in_=bl_d[:])

        # interleaved per-tile max columns: col 2t+c = (tile t, hidden chunk c)
        maxc = cpool.tile([P, NC2pad], f32)
        nc.vector.memset(maxc[:], -1.0e30)

        psum_sum = apool.tile([P, H], f32)
        psum_att = apool.tile([P, H], f32)

        for ts in range(0, NT, S_TILES):
            sn = min(S_TILES, NT - ts)
            xsuper = xpool.tile([P, S_TILES, H], bf16)
            nc.sync.dma_start(out=xsuper[:, :sn, :], in_=x_d[:, ts : ts + sn, :])
            for s2 in range(0, sn, 2):
                t = ts + s2
                x2 = xsuper[:, s2 : s2 + 2, :]  # [P, 2, H]

                # attention scores for the tile pair: one fused mult +
                # one fused reduce
                trash2 = wpool.tile([P, 2, H], bf16)
                scores2 = wpool.tile([P, 2], f32)
                nc.vector.tensor_tensor(
                    out=trash2[:],
                    in0=x2,
                    in1=wrow2[:].rearrange("p (s h) -> p s h", s=2),
                    op=mybir.AluOpType.mult,
                )
                nc.vector.tensor_reduce(
                    scores2[:],
                    trash2[:],
                    axis=mybir.AxisListType.X,
                    op=mybir.AluOpType.add,
                )
                sig2 = wpool.tile([P, 2], f32)
                nc.scalar.activation(
                    sig2[:],
                    scores2[:],
                    mybir.ActivationFunctionType.Sigmoid,
                    bias=bcol[:, 0:1],
                    scale=1.0,
                )

                # transposes for the pair into one PSUM tile:
                # slot 2s+c = (tile s-in-pair, hidden chunk c)
                ptg = tpool.tile([P, 4, P], bf16)

                for s in range(2):
                    tt = t + s
                    xt = xsuper[:, s2 + s, :]

                    onehot = wpool.tile([P, P], bf16)
                    nc.vector.tensor_scalar(
                        out=onehot[:],
                        in0=iota[:],
                        scalar1=bl[:, tt : tt + 1],
                        scalar2=None,
                        op0=mybir.AluOpType.is_equal,
                    )
                    onehot_sig = wpool.tile([P, P], bf16)
                    nc.scalar.mul(onehot_sig[:], onehot[:], sig2[:, s : s + 1])

                    first = tt == 0
                    last = tt == NT - 1
                    nc.tensor.matmul(
                        psum_sum[:], lhsT=onehot[:], rhs=xt, start=first, stop=last
                    )
                    nc.tensor.matmul(
                        psum_att[:], lhsT=onehot_sig[:], rhs=xt,
                        start=first, stop=last,
                    )
                    nc.tensor.transpose(ptg[:, 2 * s, :], xt[:, 0:P], ident[:])
                    nc.tensor.transpose(
                        ptg[:, 2 * s + 1, :], xt[:, P : 2 * P], ident[:]
                    )

                # one fused max reduce for the pair -> 4 interleaved columns
                nc.vector.tensor_reduce(
                    maxc[:, 2 * t : 2 * t + 4],
                    ptg[:],
                    axis=mybir.AxisListType.X,
                    op=mybir.AluOpType.max,
                )

        # ---- epilogue ----
        bias_sb = {}
        for k in ks:
            bias_sb[k] = cpool.tile(
                [P, 2 * NT], f32, name=f"bias{k}", tag=f"bias{k}"
            )
            nc.sync.dma_start(out=bias_sb[k][:], in_=bias_d[k][:])
        ohm0 = cpool.tile([P, KC2, P], f32)
        nc.sync.dma_start(out=ohm0[:], in_=ohm0_d[:])
        ohm1 = cpool.tile([P, KC2, P], f32)
        nc.sync.dma_start(out=ohm1[:], in_=ohm1_d[:])
        invcnt = cpool.tile([P, 1], f32)
        nc.sync.dma_start(out=invcnt[:], in_=invcnt_d[:])

        # masked max tournament over interleaved columns (shift 2k)
        for k in ks:
            if k >= NT:
                break
            w2 = 2 * (NT - k)
            tmp = wpool.tile([P, NC2pad], f32, tag="tmp_tourn")
            nc.vector.tensor_tensor(
                out=tmp[:, 0:w2],
                in0=maxc[:, 2 * k : 2 * NT],
                in1=bias_sb[k][:, 0:w2],
                op=mybir.AluOpType.add,
            )
            nc.vector.tensor_tensor(
                out=maxc[:, 0:w2],
                in0=maxc[:, 0:w2],
                in1=tmp[:, 0:w2],
                op=mybir.AluOpType.max,
            )

        # transpose interleaved max columns to (tile,chunk)-major rows and
        # extract per-segment max: chunk-0 rows -> out[:, 0:128],
        # chunk-1 rows -> out[:, 128:256]
        psum_max0 = apool.tile([P, P], f32)
        psum_max1 = apool.tile([P, P], f32)
        identf = cpool.tile([P, P], f32)
        nc.vector.tensor_copy(identf[:], ident[:])
        for kc in range(KC2):
            ptm = tpool.tile([P, P], f32, tag="ptm")
            nc.tensor.transpose(
                ptm[:], maxc[:, kc * P : (kc + 1) * P], identf[:]
            )
            tmt = wpool.tile([P, P], f32, tag="tmt")
            nc.scalar.copy(tmt[:], ptm[:])
            nc.tensor.matmul(
                psum_max0[:],
                lhsT=ohm0[:, kc, :],
                rhs=tmt[:],
                start=(kc == 0),
                stop=(kc == KC2 - 1),
            )
            nc.tensor.matmul(
                psum_max1[:],
                lhsT=ohm1[:, kc, :],
                rhs=tmt[:],
                start=(kc == 0),
                stop=(kc == KC2 - 1),
            )

        out_sb = cpool.tile([P, 3 * H], f32)
        nc.scalar.mul(out_sb[:, 0:H], psum_sum[:], invcnt[:, 0:1])
        nc.scalar.copy(out_sb[:, H : H + P], psum_max0[:])
        nc.scalar.copy(out_sb[:, H + P : 2 * H], psum_max1[:])
        nc.scalar.copy(out_sb[:, 2 * H : 3 * H], psum_att[:])
        nc.sync.dma_start(out=out_d[:], in_=out_sb[:])

    nc.finalize()
    return nc


def _prepare_inputs(x, batch, att_w, att_b):
    """Host-side sharding/index preprocessing. Returns (in_maps, NT, KC, ks)."""
    N = x.shape[0]
    assert x.shape == (N, H) and batch.shape == (N,)

    counts = np.bincount(batch, minlength=G).astype(np.int64)
    starts = np.concatenate([[0], np.cumsum(counts)])
    tiles_per_seg = (counts + P - 1) // P  # 0 for empty segments

    core_nt = [
        int(tiles_per_seg[c * SEGS_PER_CORE : (c + 1) * SEGS_PER_CORE].sum())
        for c in range(CORES)
    ]
    NT = max(max(core_nt), 2)
    NT = ((NT + S_TILES - 1) // S_TILES) * S_TILES  # pad to super-tile multiple
    KC = (NT + P - 1) // P
    KC2 = (2 * NT + P - 1) // P
    NC2pad = KC2 * P

    max_run = int(tiles_per_seg.max())
    ks = []
    k = 1
    while k < max(max_run, 1):
        ks.append(k)
        k *= 2
    if not ks:
        ks = [1]

    iota_mat = _bf16(np.tile(np.arange(P, dtype=np.float32), (P, 1)))
    ident = _bf16(np.eye(P, dtype=np.float32))
    wrow = np.tile(att_w.reshape(1, H), (P, 1)).astype(np.float32)
    wrow2 = _bf16(np.tile(wrow, (1, 2)))
    bcol = np.full((P, 1), att_b[0], dtype=np.float32)

    in_maps = []
    for c in range(CORES):
        g0 = c * SEGS_PER_CORE
        flat_x = np.full((NT * P, H), PAD_X, dtype=np.float32)
        flat_bl = np.full((NT * P,), float(P), dtype=np.float32)
        seg_of_tile = np.full((NT,), -1, dtype=np.int64)
        ohm0 = np.zeros((NC2pad, P), dtype=np.float32)
        ohm1 = np.zeros((NC2pad, P), dtype=np.float32)

        t = 0
        for gl in range(SEGS_PER_CORE):
            g = g0 + gl
            cnt = int(counts[g])
            if cnt == 0:
                continue
            ntg = int(tiles_per_seg[g])
            n0 = int(starts[g])
            flat_x[t * P : t * P + cnt] = x[n0 : n0 + cnt]
            flat_bl[t * P : t * P + cnt] = float(gl)
            seg_of_tile[t : t + ntg] = gl
            ohm0[2 * t, gl] = 1.0
            ohm1[2 * t + 1, gl] = 1.0
            t += ntg

        x_dev = _bf16(flat_x.reshape(NT, P, H).transpose(1, 0, 2))
        bl_dev = flat_bl.reshape(NT, P).T.astype(np.float32)

        m = {
            "x": np.ascontiguousarray(x_dev),
            "bl": np.ascontiguousarray(bl_dev),
            "wrow2": wrow2,
            "bcol": bcol,
            "iota": iota_mat,
            "ident": ident,
            "ohm0": np.ascontiguousarray(
                ohm0.reshape(KC2, P, P).transpose(1, 0, 2)
            ),
            "ohm1": np.ascontiguousarray(
                ohm1.reshape(KC2, P, P).transpose(1, 0, 2)
            ),
            "invcnt": (
                1.0
                / np.maximum(counts[g0 : g0 + SEGS_PER_CORE], 1).astype(np.float32)
            ).reshape(P, 1),
        }
        for k in ks:
            bias = np.full((P, 2 * NT), NEG_BIG, dtype=np.float32)
            same = (seg_of_tile[k:] == seg_of_tile[:-k]) & (seg_of_tile[:-k] >= 0)
            same2 = np.repeat(same, 2)
            bias[:, : 2 * (NT - k)][:, same2] = 0.0
            m[f"bias{k}"] = bias
        in_maps.append(m)

    return in_maps, NT, KC, ks


def kernel(x, batch, att_w, att_b):
    x = np.ascontiguousarray(np.asarray(x, dtype=np.float32))
    batch = np.asarray(batch).astype(np.int64)
    att_w = np.asarray(att_w, dtype=np.float32).reshape(H, 1)
    att_b = np.asarray(att_b, dtype=np.float32).reshape(1)

    in_maps, NT, KC, ks = _prepare_inputs(x, batch, att_w, att_b)

    # ---- compile (cached) and run ----
    key = (NT, KC, tuple(ks))
    if key not in _compiled_cache:
        _compiled_cache[key] = _build_program(NT, KC, ks)
    nc = _compiled_cache[key]

    from concourse.bass_utils import run_bass_kernel_spmd

    res = run_bass_kernel_spmd(nc, in_maps, list(range(CORES)))
    global _last_result
    _last_result = res
    out = np.concatenate(
        [np.asarray(res.results[c]["out"]) for c in range(CORES)], axis=0
    )
    return out.astype(np.float32)
